# revision 54
# baseline (speedup 1.0000x reference)
"""Trainium2 Bass kernel for the LocalAggregator nn.Module.

Reference computation:
    power[p,g]  = -0.5 * d^T Prec_g d          (d = pts[p] - means3D[g])
    within[p,g] = all(|voxel(pts[p]) - voxel(means3D[g])| <= radii[g])
    logits      = where(within & power<=0, exp(power), 0) @ opacities

Device algorithm (everything O(P*G) runs on the NeuronCores):
  * Points are split into 64 spatial blocks of 256 (k-d median splits);
    each block only interacts with the gaussians whose voxel box
    reaches one of the block's points (~50-130 of 2048), found exactly
    on the host in O(P+G) per block.
  * Per (block, 128-gaussian chunk) job, ONE fp16 matmul of K<=128
    feature rows computes power + box penalty into PSUM fp32:
      - the quadratic form is expanded around the block center and
        every (feature, coefficient) product is split hi/lo into fp16
        pairs (3 rows per term -> ~2^-22 relative error),
      - the voxel box test contributes 224*(within_a - 1) per axis via
        one-hot rows over the DISTINCT voxel values of the block's
        points (compressed: clustered data needs <= ~32 rows), so
        out-of-box pairs get power <= -224 and exp underflows to +0.0
        in fp32, exactly reproducing the reference's hard mask.
  * ScalarE evaluates exp (batched (2,4,2) jobs per instruction so the
    serial exp chain starts as early as the first DMA allows),
    TensorE contracts the fp16 weights against opacities, and the
    [C, 256] logits accumulate in PSUM per block.
  * The PE is warmed up with two dummy matmuls at t~0 so the clock
    ramp (HAM) completes before the bulk of the matmuls run.
  * Tail: logits drain through 4 quarter PSUM tiles into fp16 staging
    (host casts back to fp32) via gate-aligned copies balanced across
    DVE and ACT (s4 on ACT ahead of s67, s5 on DVE after s23), then
    two output DMAs whose chains are co-optimal with the copy gates;
    the hot input DMA only moves the partition rows the first four
    blocks actually use.

Sharding: 8 blocks per core (greedy-balanced by chunk count); host
does only O(P log P + blocks*G) prep and the final permutation
scatter of the [P, C] output.
"""

import numpy as np

import concourse.bass as bass
import concourse.mybir as mybir
import concourse.tile as tile
import concourse.bass2jax as _bass2jax
import concourse.bass_utils as _bass_utils
from concourse.bass_utils import run_bass_kernel_spmd

import json as _json


def _split_waits(bir_json):
    """Walrus in this toolchain rejects instructions carrying more than one
    sync wait ("Too many sync wait commands").  Split every multi-wait
    instruction into a chain of single-wait NoOps on the same engine (program
    order on the engine's sequencer preserves the wait-before-op semantics)."""
    if isinstance(bir_json, (bytes, bytearray)):
        m = _json.loads(bir_json.decode())
    else:
        m = _json.loads(bir_json)
    cnt = 0
    for f in m["functions"]:
        for bb in f["blocks"]:
            new_insts = []
            for inst in bb["instructions"]:
                si = inst.get("sync_info")
                waits = (si or {}).get("on_wait") or []
                if len(waits) > 1:
                    eng = inst.get("engine")
                    for w in waits[:-1]:
                        cnt += 1
                        nop = {
                            "debug": 16,
                            "ins": [],
                            "name": f"I-nopw-{cnt}",
                            "opcode": "NoOp",
                            "outs": [],
                            "sync_info": {"on_update": [], "on_wait": [w]},
                        }
                        if eng is not None:
                            nop["engine"] = eng
                        new_insts.append(nop)
                    si["on_wait"] = [waits[-1]]
                new_insts.append(inst)
            bb["instructions"] = new_insts
    return _json.dumps(m).encode()


_orig_compile_bir_kernel = _bass_utils.compile_bir_kernel.__wrapped__ if hasattr(
    _bass_utils.compile_bir_kernel, "__wrapped__") else _bass_utils.compile_bir_kernel


def _patched_compile_bir_kernel(bir_json, tmpdir, neff_name="file.neff"):
    return _orig_compile_bir_kernel(_split_waits(bir_json), tmpdir, neff_name)


_bass2jax.compile_bir_kernel = _patched_compile_bir_kernel
_bass_utils.compile_bir_kernel = _patched_compile_bir_kernel

GRID = np.float32(0.5)
SCALE_MULT = np.float32(3.0)
MPEN = 224.0  # per-axis box penalty; exact in fp16, 3*224 >> 104 (fp32 exp underflow)
N_CORES = 8
NSLICE = 8  # point blocks (slices) per core
BLK = 256  # points per block
NPOLY = 20  # fp16 hi/lo polynomial rows (diagonal precision matrices)
WARM_N = 2  # tiny early matmuls start the PE pstate-ramp clock at t~0
WARM_FREE = 256

_nc_cache = {}


def _build_bass(KT, NJOBS, C, HR):
    """KT: contraction rows (poly + max one-hot); NJOBS: jobs (block-chunks)
    per core, jobs 0..7 -> slices 0..7, jobs >=8 -> slice 7 extras; HR:
    rows actually used by slices 0-3 (low-row blocks sorted first), so the
    critical first DMA moves fewer bytes."""
    f16 = mybir.dt.float16
    f32 = mybir.dt.float32
    HOT = 4 * 128 + 4 * 256  # first DMA: W jobs 0-3 + F slices 0-3
    TOTC = NJOBS * 128 + NSLICE * BLK

    def col_w(j):
        return j * 128 if j < 4 else HOT + (j - 4) * 128

    def col_f(s):
        return 512 + s * BLK if s < 4 else HOT + (NJOBS - 4) * 128 + (s - 4) * BLK

    def slice_of(j):
        return j if j < NSLICE else NSLICE - 1

    nc = bass.Bass()
    fw_d = nc.dram_tensor("fw", [KT, TOTC], f16, kind="ExternalInput")
    opa_d = nc.dram_tensor("opa", [128, NJOBS * C], f16, kind="ExternalInput")
    # fp16 output staging halves the tail DMA; host casts back to fp32
    # (adds ~2^-11 relative error, ~100x under tolerance)
    out_d = nc.dram_tensor("out", [C, NSLICE * BLK], f16, kind="ExternalOutput")

    # job groups sharing one PSUM tile + one exp instruction: a small first
    # group starts the serial exp chain as early as possible (tuned (2,4,2)
    # for NJOBS=8; generic tail of <=4 otherwise)
    if NJOBS == 8:
        groups = [[0, 1], [2, 3, 4, 5], [6, 7]]
    else:
        groups = [list(range(g, min(g + 4, NJOBS))) for g in range(0, NJOBS, 4)]

    with tile.TileContext(nc) as tc:
        with (
            tc.tile_pool(name="singles", bufs=1) as singles,
            tc.tile_pool(name="wt", bufs=2) as wtp,
            tc.tile_pool(name="osb", bufs=1) as osbp,
            tc.tile_pool(name="pp", bufs=2, space="PSUM") as pp,
            tc.tile_pool(name="pl", bufs=1, space="PSUM") as pl,
        ):
            # --- PE warm-up: memset a scratch tile, then dummy matmuls ---
            warm_sb = singles.tile([KT, max(128, WARM_FREE)], f16)
            nc.vector.memset(warm_sb[:], 0.0)
            warm_ps = pp.tile([128, 4 * BLK], f32, name="ps")
            for i in range(WARM_N):
                nc.tensor.matmul(
                    warm_ps[:, :WARM_FREE], warm_sb[:, :128],
                    warm_sb[:, :WARM_FREE], start=True, stop=True,
                )

            # --- inputs ---
            fw_sb = singles.tile([KT, TOTC], f16)
            opa_sb = singles.tile([128, NJOBS * C], f16)
            nc.sync.dma_start(out=fw_sb[:HR, :HOT], in_=fw_d[:HR, :HOT])
            nc.sync.dma_start(out=fw_sb[:, HOT:], in_=fw_d[:, HOT:])
            nc.sync.dma_start(out=opa_sb[:], in_=opa_d[:])

            # --- job groups: all power matmuls + exp first (keeps the
            # serial ACT chain dense), then every logits matmul ---
            wts = []
            ltile = {}
            for gi, jobs in enumerate(groups):
                gw = BLK * len(jobs)
                pp_t = pp.tile([128, gw], f32, name="ps")
                for k, j in enumerate(jobs):
                    kj = HR if j < 4 else KT  # hot jobs only use rows < HR
                    nc.tensor.matmul(
                        pp_t[:, k * BLK:(k + 1) * BLK],
                        fw_sb[:kj, col_w(j):col_w(j) + 128],
                        fw_sb[:kj, col_f(slice_of(j)):col_f(slice_of(j)) + BLK],
                        start=True, stop=True,
                    )
                wt_t = wtp.tile([128, gw], f16, name="wt")
                nc.scalar.activation(
                    out=wt_t[:], in_=pp_t[:], func=mybir.ActivationFunctionType.Exp
                )
                wts.append((jobs, wt_t))
            for pend in wts:
                _emit_logits(nc, pl, ltile, pend, opa_sb, C, NJOBS)

            # --- drain logits PSUM -> SBUF -> DRAM ---
            # 4 quarter tiles with copies balanced across DVE (free early)
            # and ACT (free after the last exp); 2 output DMAs, the first
            # hidden under the tail's copies.
            o0 = osbp.tile([C, 4 * BLK], f16, name="o0")
            o1 = osbp.tile([C, 4 * BLK], f16, name="o1")
            nc.vector.tensor_copy(out=o0[:, :2 * BLK], in_=ltile[0][:])
            nc.vector.tensor_copy(out=o0[:, 2 * BLK:], in_=ltile[1][:])
            # s4 goes to ACT ahead of c67 (its data is ready first), s5 to
            # DVE after c23 — keeps both engines' last copies at their gates
            nc.scalar.activation(
                out=o1[:, :BLK], in_=ltile[2][:, :BLK],
                func=mybir.ActivationFunctionType.Copy,
            )
            nc.vector.tensor_copy(out=o1[:, BLK:2 * BLK], in_=ltile[2][:, BLK:])
            nc.scalar.activation(
                out=o1[:, 2 * BLK:], in_=ltile[3][:],
                func=mybir.ActivationFunctionType.Copy,
            )
            nc.sync.dma_start(out=out_d[:, :4 * BLK], in_=o0[:])
            nc.sync.dma_start(out=out_d[:, 4 * BLK:], in_=o1[:])
    return nc


def _emit_logits(nc, pl, ltile, pend, opa_sb, C, njobs):
    jobs, wt_t = pend
    for k, j in enumerate(jobs):
        s = j if j < NSLICE else NSLICE - 1
        q = s // 2
        if q not in ltile:
            ltile[q] = pl.tile([C, 2 * BLK], mybir.dt.float32, name=f"pl{q}")
        # slice 7 accumulates its extra-chunk jobs; others are single matmuls
        last_j = njobs - 1 if s == NSLICE - 1 else j
        nc.tensor.matmul(
            ltile[q][:, (s % 2) * BLK:(s % 2 + 1) * BLK],
            opa_sb[:, j * C:(j + 1) * C],
            wt_t[:, k * BLK:(k + 1) * BLK],
            start=(j == s), stop=(j == last_j),
        )


def _hilo(v):
    vh = v.astype(np.float16)
    vl = (v - vh.astype(np.float64)).astype(np.float16)
    return vh, vl


def _kd_split(pts, pts_int, idx, depth, gsel_count):
    """Median k-d split; the last two levels pick the axis combination that
    minimizes the worst per-block gaussian count (keeps every block <= 128
    gaussians so each needs exactly one 128-column chunk)."""
    def split(ix, ax):
        order = np.argsort(pts[ix, ax], kind="stable")
        half = len(ix) // 2
        return ix[order[:half]], ix[order[half:]]

    if depth == 2:
        best = None
        for a0 in range(3):
            l, r = split(idx, a0)
            for al in range(3):
                ll = split(l, al)
                for ar in range(3):
                    leaves = [*ll, *split(r, ar)]
                    gs = [gsel_count(x) for x in leaves]
                    keyv = (max(gs), sum(int(np.ceil(max(g, 1) / 128)) for g in gs))
                    if best is None or keyv < best[0]:
                        best = (keyv, leaves)
        return best[1]
    pi = pts_int[idx]
    ax = int(np.argmax(pi.max(0) - pi.min(0)))
    l, r = split(idx, ax)
    return (_kd_split(pts, pts_int, l, depth - 1, gsel_count)
            + _kd_split(pts, pts_int, r, depth - 1, gsel_count))


def _prepare(inputs):
    """Host-side O(P log P + blocks*G) prep: blocking, gaussian selection,
    fp16 feature/coefficient matrices."""
    pts = np.ascontiguousarray(np.asarray(inputs["pts"], dtype=np.float32))
    means3D = np.ascontiguousarray(np.asarray(inputs["means3D"], dtype=np.float32))
    opac = np.asarray(inputs["opacities"], dtype=np.float32)
    scales = np.asarray(inputs["scales"], dtype=np.float32)
    cov3D = np.asarray(inputs["cov3D"], dtype=np.float32)
    pc_min = np.asarray(inputs["pc_min"], dtype=np.float32)

    P = pts.shape[0]
    G = means3D.shape[0]
    C = opac.shape[1]
    NBLK = P // BLK
    assert NBLK == N_CORES * NSLICE, (P, BLK)

    # integer voxel quantities, identical fp32 arithmetic to the reference
    pts_int = np.floor((pts - pc_min[None, :]) / GRID).astype(np.int32)
    means_int = np.floor((means3D - pc_min[None, :]) / GRID).astype(np.int32)
    radii = np.ceil(scales.max(-1) * SCALE_MULT / GRID).astype(np.int32)
    a_diag = np.stack([cov3D[:, 0, 0], cov3D[:, 1, 1], cov3D[:, 2, 2]], 1).astype(np.float64)
    off = cov3D.reshape(G, 9)[:, [1, 5, 2]]
    assert np.abs(off).max() == 0.0, "non-diagonal cov3D unsupported by this kernel"

    def gsel_exact(ix):
        """Gaussians with at least one of the points inside their voxel box."""
        pi = pts_int[ix]
        cand = np.where(((means_int >= pi.min(0) - radii[:, None])
                         & (means_int <= pi.max(0) + radii[:, None])).all(1))[0]
        within = (np.abs(pi[:, None, :] - means_int[None, cand, :])
                  <= radii[cand][None, :, None]).all(-1).any(0)
        return cand[within]

    blocks = _kd_split(pts, pts_int, np.arange(P), 6,
                       lambda ix: len(gsel_exact(ix)))

    # per block: compressed one-hot rows + exact gaussian selection
    binfo = []
    for b in blocks:
        pi = pts_int[b]
        uniq = [np.unique(pi[:, a]) for a in range(3)]
        nrows = sum(len(u) for u in uniq)
        binfo.append((b, uniq, gsel_exact(b)))
        assert NPOLY + nrows <= 128, f"row budget exceeded: {NPOLY + nrows}"

    KT = max(NPOLY + sum(len(u) for u in info[1]) for info in binfo)
    chunks = [max(1, int(np.ceil(len(info[2]) / 128))) for info in binfo]

    nrows_all = [NPOLY + sum(len(u) for u in info[1]) for info in binfo]
    if all(c == 1 for c in chunks):
        # one chunk per block: deal the globally lowest-row blocks into the
        # hot slots (4 per core) so HR (the critical first DMA's row count)
        # is the 32nd-lowest row count instead of an assignment accident
        order = np.argsort(nrows_all, kind="stable")
        core_blocks = [[] for _ in range(N_CORES)]
        for k, bi in enumerate(order):
            core_blocks[k % N_CORES].append(int(bi))
        core_chunks = [NSLICE] * N_CORES
    else:
        # greedy block->core assignment balancing chunk counts
        order = np.argsort(-np.asarray(chunks), kind="stable")
        core_blocks = [[] for _ in range(N_CORES)]
        core_chunks = [0] * N_CORES
        for bi in order:
            ci = min((c for c in range(N_CORES) if len(core_blocks[c]) < NSLICE),
                     key=lambda c: core_chunks[c])
            core_blocks[ci].append(bi)
            core_chunks[ci] += chunks[bi]
    NJOBS = max(core_chunks)
    assert NJOBS >= NSLICE

    HOT = 4 * 128 + 4 * 256
    TOTC = NJOBS * 128 + NSLICE * BLK

    def col_w(j):
        return j * 128 if j < 4 else HOT + (j - 4) * 128

    def col_f(s):
        return 512 + s * BLK if s < 4 else HOT + (NJOBS - 4) * 128 + (s - 4) * BLK

    in_maps = []
    perm = np.empty((N_CORES, NSLICE * BLK), np.int64)
    nrows_of = {bi: NPOLY + sum(len(u) for u in binfo[bi][1]) for bi in range(len(binfo))}
    HR = 0
    for ci in range(N_CORES):
        # low-row blocks first (shrinks the hot DMA), multi-chunk block last
        blks = sorted(core_blocks[ci], key=lambda bi: (chunks[bi], nrows_of[bi]))
        core_blocks[ci] = blks
        HR = max(HR, max(nrows_of[bi] for bi in blks[:4]))
    for ci in range(N_CORES):
        blks = core_blocks[ci]
        assert sum(c > 1 for c in (chunks[bi] for bi in blks[:-1])) == 0, \
            "more than one multi-chunk block on a core"
        FW = np.zeros((KT, TOTC), np.float16)
        OPA = np.zeros((128, NJOBS * C), np.float16)
        job = 0
        for si, bi in enumerate(blks):
            b, uniq, sel = binfo[bi]
            perm[ci, si * BLK:(si + 1) * BLK] = b
            pi = pts_int[b]
            lo = pi.min(0)
            hi = pi.max(0)
            cen = (lo + hi + 1).astype(np.float64) * (0.5 * float(GRID))
            p64 = pts[b].astype(np.float64) - cen

            # --- features for this slice ---
            F = np.zeros((KT, BLK), np.float16)
            r = 0
            for ax in range(3):
                qh, ql = _hilo(p64[:, ax] ** 2)
                xh, xl = _hilo(p64[:, ax])
                F[r], F[r + 1], F[r + 2] = qh, ql, qh
                F[r + 3], F[r + 4], F[r + 5] = xh, xl, xh
                r += 6
            F[18] = np.float16(1.0)
            F[19] = np.float16(1.0)
            offs = []
            racc = NPOLY
            for ax in range(3):
                offs.append(racc)
                racc += len(uniq[ax])
            tcol = np.arange(BLK)
            for ax in range(3):
                rank = np.searchsorted(uniq[ax], pi[:, ax])
                F[offs[ax] + rank, tcol] = np.float16(1.0)
            FW[:, col_f(si):col_f(si) + BLK] = F

            # --- per-chunk gaussian coefficients ---
            nch = chunks[bi]
            for ch in range(nch):
                gsel = sel[ch * 128:(ch + 1) * 128]
                gl = len(gsel)
                m64 = means3D[gsel].astype(np.float64) - cen
                a = a_diag[gsel]
                W = np.zeros((KT, 128), np.float16)
                r = 0
                for ax in range(3):
                    wah, wal = _hilo(-0.5 * a[:, ax])
                    wbh, wbl = _hilo(a[:, ax] * m64[:, ax])
                    W[r, :gl], W[r + 1, :gl], W[r + 2, :gl] = wah, wah, wal
                    W[r + 3, :gl], W[r + 4, :gl], W[r + 5, :gl] = wbh, wbh, wbl
                    r += 6
                ch_, cl_ = _hilo(-0.5 * (a * m64 ** 2).sum(1))
                W[18, :gl], W[19, :gl] = ch_, cl_
                for ax in range(3):
                    u = uniq[ax]
                    box = ((u[:, None] >= (means_int[gsel, ax] - radii[gsel])[None, :])
                           & (u[:, None] <= (means_int[gsel, ax] + radii[gsel])[None, :]))
                    W[offs[ax]:offs[ax] + len(u), :gl] = np.where(
                        box, np.float16(0.0), np.float16(-MPEN))
                if ch == 0:
                    assert job == si, (job, si)
                # jobs 0..7 occupy slots 0..7 (slice order); extra chunks of
                # the last block (slice 7) land at slots 8..
                FW[:, col_w(job):col_w(job) + 128] = W
                OPA[:gl, job * C:(job + 1) * C] = opac[gsel].astype(np.float16)
                job += 1
        in_maps.append({"fw": FW, "opa": OPA})

    return in_maps, perm, (P, KT, NJOBS, C, HR)


def _run(inputs, trace=False, **run_kwargs):
    in_maps, perm, (P, KT, NJOBS, C, HR) = _prepare(inputs)
    key = (KT, NJOBS, C, HR)
    if key not in _nc_cache:
        nc = _build_bass(KT, NJOBS, C, HR)
        _nc_cache[key] = nc
    nc = _nc_cache[key]
    try:
        res = run_bass_kernel_spmd(
            nc, in_maps, core_ids=list(range(N_CORES)), trace=trace, **run_kwargs
        )
    except ModuleNotFoundError:
        res = run_bass_kernel_spmd(
            nc, in_maps, core_ids=list(range(N_CORES)), trace=False, **run_kwargs
        )
    out = np.empty((P, C), np.float32)
    for ci in range(N_CORES):
        out[perm[ci]] = res.results[ci]["out"].T.astype(np.float32)
    return out, res


def kernel(**inputs):
    return _run(inputs)[0]


# revision 55
# speedup vs baseline: 1.0024x; 1.0024x over previous
"""Trainium2 Bass kernel for the LocalAggregator nn.Module.

Reference computation:
    power[p,g]  = -0.5 * d^T Prec_g d          (d = pts[p] - means3D[g])
    within[p,g] = all(|voxel(pts[p]) - voxel(means3D[g])| <= radii[g])
    logits      = where(within & power<=0, exp(power), 0) @ opacities

Device algorithm (everything O(P*G) runs on the NeuronCores):
  * Points are split into 64 spatial blocks of 256 (k-d median splits);
    each block only interacts with the gaussians whose voxel box
    reaches one of the block's points (~50-130 of 2048), found exactly
    on the host in O(P+G) per block.
  * Per (block, 128-gaussian chunk) job, ONE fp16 matmul of K<=128
    feature rows computes power + box penalty into PSUM fp32:
      - the quadratic form is expanded around the block center and
        every (feature, coefficient) product is split hi/lo into fp16
        pairs (3 rows per term -> ~2^-22 relative error),
      - the voxel box test contributes 224*(within_a - 1) per axis via
        one-hot rows over the DISTINCT voxel values of the block's
        points (compressed: clustered data needs <= ~32 rows), so
        out-of-box pairs get power <= -224 and exp underflows to +0.0
        in fp32, exactly reproducing the reference's hard mask.
  * ScalarE evaluates exp (batched (2,4,2) jobs per instruction so the
    serial exp chain starts as early as the first DMA allows),
    TensorE contracts the fp16 weights against opacities, and the
    [C, 256] logits accumulate in PSUM per block.
  * The PE is warmed up with two dummy matmuls at t~0 so the clock
    ramp (HAM) completes before the bulk of the matmuls run.
  * Tail: logits drain through 4 quarter PSUM tiles into fp16 staging
    (host casts back to fp32) via gate-aligned copies balanced across
    DVE and ACT (s4 on ACT ahead of s67, s5 on DVE after s23), then
    two output DMAs whose chains are co-optimal with the copy gates;
    the hot input DMA only moves the partition rows the first four
    blocks actually use.

Sharding: 8 blocks per core (greedy-balanced by chunk count); host
does only O(P log P + blocks*G) prep and the final permutation
scatter of the [P, C] output.
"""

import numpy as np

import concourse.bass as bass
import concourse.mybir as mybir
import concourse.tile as tile
import concourse.bass2jax as _bass2jax
import concourse.bass_utils as _bass_utils
from concourse.bass_utils import run_bass_kernel_spmd

import json as _json


def _split_waits(bir_json):
    """Walrus in this toolchain rejects instructions carrying more than one
    sync wait ("Too many sync wait commands").  Split every multi-wait
    instruction into a chain of single-wait NoOps on the same engine (program
    order on the engine's sequencer preserves the wait-before-op semantics)."""
    if isinstance(bir_json, (bytes, bytearray)):
        m = _json.loads(bir_json.decode())
    else:
        m = _json.loads(bir_json)
    cnt = 0
    for f in m["functions"]:
        for bb in f["blocks"]:
            new_insts = []
            for inst in bb["instructions"]:
                si = inst.get("sync_info")
                waits = (si or {}).get("on_wait") or []
                if len(waits) > 1:
                    eng = inst.get("engine")
                    for w in waits[:-1]:
                        cnt += 1
                        nop = {
                            "debug": 16,
                            "ins": [],
                            "name": f"I-nopw-{cnt}",
                            "opcode": "NoOp",
                            "outs": [],
                            "sync_info": {"on_update": [], "on_wait": [w]},
                        }
                        if eng is not None:
                            nop["engine"] = eng
                        new_insts.append(nop)
                    si["on_wait"] = [waits[-1]]
                new_insts.append(inst)
            bb["instructions"] = new_insts
    return _json.dumps(m).encode()


_orig_compile_bir_kernel = _bass_utils.compile_bir_kernel.__wrapped__ if hasattr(
    _bass_utils.compile_bir_kernel, "__wrapped__") else _bass_utils.compile_bir_kernel


def _patched_compile_bir_kernel(bir_json, tmpdir, neff_name="file.neff"):
    return _orig_compile_bir_kernel(_split_waits(bir_json), tmpdir, neff_name)


_bass2jax.compile_bir_kernel = _patched_compile_bir_kernel
_bass_utils.compile_bir_kernel = _patched_compile_bir_kernel

GRID = np.float32(0.5)
SCALE_MULT = np.float32(3.0)
MPEN = 224.0  # per-axis box penalty; exact in fp16, 3*224 >> 104 (fp32 exp underflow)
N_CORES = 8
NSLICE = 8  # point blocks (slices) per core
BLK = 256  # points per block
NPOLY = 20  # fp16 hi/lo polynomial rows (diagonal precision matrices)
WARM_N = 2  # tiny early matmuls start the PE pstate-ramp clock at t~0
WARM_FREE = 256

_nc_cache = {}


def _build_bass(KT, NJOBS, C, HR):
    """KT: contraction rows (poly + max one-hot); NJOBS: jobs (block-chunks)
    per core, jobs 0..7 -> slices 0..7, jobs >=8 -> slice 7 extras; HR:
    rows actually used by slices 0-3 (low-row blocks sorted first), so the
    critical first DMA moves fewer bytes."""
    f16 = mybir.dt.float16
    f32 = mybir.dt.float32
    HOT = 4 * 128 + 4 * 256  # first DMA: W jobs 0-3 + F slices 0-3
    TOTC = NJOBS * 128 + NSLICE * BLK

    def col_w(j):
        return j * 128 if j < 4 else HOT + (j - 4) * 128

    def col_f(s):
        return 512 + s * BLK if s < 4 else HOT + (NJOBS - 4) * 128 + (s - 4) * BLK

    def slice_of(j):
        return j if j < NSLICE else NSLICE - 1

    nc = bass.Bass()
    fw_d = nc.dram_tensor("fw", [KT, TOTC], f16, kind="ExternalInput")
    opa_d = nc.dram_tensor("opa", [128, NJOBS * C], f16, kind="ExternalInput")
    # fp16 output staging halves the tail DMA; host casts back to fp32
    # (adds ~2^-11 relative error, ~100x under tolerance)
    out_d = nc.dram_tensor("out", [C, NSLICE * BLK], f16, kind="ExternalOutput")

    # job groups sharing one PSUM tile + one exp instruction: a small first
    # group starts the serial exp chain as early as possible (tuned (2,4,2)
    # for NJOBS=8; generic tail of <=4 otherwise)
    if NJOBS == 8:
        groups = [[0, 1], [2, 3, 4, 5], [6, 7]]
    else:
        groups = [list(range(g, min(g + 4, NJOBS))) for g in range(0, NJOBS, 4)]

    with tile.TileContext(nc) as tc:
        with (
            tc.tile_pool(name="singles", bufs=1) as singles,
            tc.tile_pool(name="wt", bufs=2) as wtp,
            tc.tile_pool(name="osb", bufs=1) as osbp,
            tc.tile_pool(name="pp", bufs=2, space="PSUM") as pp,
            tc.tile_pool(name="pl", bufs=1, space="PSUM") as pl,
        ):
            # --- PE warm-up: memset a scratch tile, then dummy matmuls ---
            warm_sb = singles.tile([KT, max(128, WARM_FREE)], f16)
            nc.vector.memset(warm_sb[:], 0.0)
            warm_ps = pp.tile([128, 4 * BLK], f32, name="ps")
            for i in range(WARM_N):
                nc.tensor.matmul(
                    warm_ps[:, :WARM_FREE], warm_sb[:, :128],
                    warm_sb[:, :WARM_FREE], start=True, stop=True,
                )

            # --- inputs ---
            fw_sb = singles.tile([KT, TOTC], f16)
            opa_sb = singles.tile([128, NJOBS * C], f16)
            nc.sync.dma_start(out=fw_sb[:HR, :HOT], in_=fw_d[:HR, :HOT])
            nc.sync.dma_start(out=fw_sb[:, HOT:], in_=fw_d[:, HOT:])
            nc.sync.dma_start(out=opa_sb[:], in_=opa_d[:])

            # --- job groups: all power matmuls + exp first (keeps the
            # serial ACT chain dense), then every logits matmul ---
            wts = []
            ltile = {}
            for gi, jobs in enumerate(groups):
                gw = BLK * len(jobs)
                pp_t = pp.tile([128, gw], f32, name="ps")
                for k, j in enumerate(jobs):
                    kj = HR if j < 4 else KT  # hot jobs only use rows < HR
                    nc.tensor.matmul(
                        pp_t[:, k * BLK:(k + 1) * BLK],
                        fw_sb[:kj, col_w(j):col_w(j) + 128],
                        fw_sb[:kj, col_f(slice_of(j)):col_f(slice_of(j)) + BLK],
                        start=True, stop=True,
                    )
                wt_t = wtp.tile([128, gw], f16, name="wt")
                nc.scalar.activation(
                    out=wt_t[:], in_=pp_t[:], func=mybir.ActivationFunctionType.Exp
                )
                wts.append((jobs, wt_t))
            for pend in wts:
                _emit_logits(nc, pl, ltile, pend, opa_sb, C, NJOBS)

            # --- drain logits PSUM -> SBUF -> DRAM ---
            # 4 quarter tiles with copies balanced across DVE (free early)
            # and ACT (free after the last exp); 2 output DMAs, the first
            # hidden under the tail's copies.
            o0 = osbp.tile([C, 4 * BLK], f16, name="o0")
            o1 = osbp.tile([C, 4 * BLK], f16, name="o1")
            nc.vector.tensor_copy(out=o0[:, :2 * BLK], in_=ltile[0][:])
            nc.vector.tensor_copy(out=o0[:, 2 * BLK:], in_=ltile[1][:])
            # s4 goes to ACT ahead of c67 (its data is ready first), s5 to
            # DVE after c23 — keeps both engines' last copies at their gates
            nc.scalar.activation(
                out=o1[:, :BLK], in_=ltile[2][:, :BLK],
                func=mybir.ActivationFunctionType.Copy,
            )
            nc.vector.tensor_copy(out=o1[:, BLK:2 * BLK], in_=ltile[2][:, BLK:])
            nc.scalar.activation(
                out=o1[:, 2 * BLK:], in_=ltile[3][:],
                func=mybir.ActivationFunctionType.Copy,
            )
            # o0's DMA goes through the gpsimd SWDGE queue so its sequencer
            # hold never delays o1's (critical) DMA on the SP queue
            nc.gpsimd.dma_start(out=out_d[:, :4 * BLK], in_=o0[:])
            nc.sync.dma_start(out=out_d[:, 4 * BLK:], in_=o1[:])
    return nc


def _emit_logits(nc, pl, ltile, pend, opa_sb, C, njobs):
    jobs, wt_t = pend
    for k, j in enumerate(jobs):
        s = j if j < NSLICE else NSLICE - 1
        q = s // 2
        if q not in ltile:
            ltile[q] = pl.tile([C, 2 * BLK], mybir.dt.float32, name=f"pl{q}")
        # slice 7 accumulates its extra-chunk jobs; others are single matmuls
        last_j = njobs - 1 if s == NSLICE - 1 else j
        nc.tensor.matmul(
            ltile[q][:, (s % 2) * BLK:(s % 2 + 1) * BLK],
            opa_sb[:, j * C:(j + 1) * C],
            wt_t[:, k * BLK:(k + 1) * BLK],
            start=(j == s), stop=(j == last_j),
        )


def _hilo(v):
    vh = v.astype(np.float16)
    vl = (v - vh.astype(np.float64)).astype(np.float16)
    return vh, vl


def _kd_split(pts, pts_int, idx, depth, gsel_count):
    """Median k-d split; the last two levels pick the axis combination that
    minimizes the worst per-block gaussian count (keeps every block <= 128
    gaussians so each needs exactly one 128-column chunk)."""
    def split(ix, ax):
        order = np.argsort(pts[ix, ax], kind="stable")
        half = len(ix) // 2
        return ix[order[:half]], ix[order[half:]]

    if depth == 2:
        best = None
        for a0 in range(3):
            l, r = split(idx, a0)
            for al in range(3):
                ll = split(l, al)
                for ar in range(3):
                    leaves = [*ll, *split(r, ar)]
                    gs = [gsel_count(x) for x in leaves]
                    keyv = (max(gs), sum(int(np.ceil(max(g, 1) / 128)) for g in gs))
                    if best is None or keyv < best[0]:
                        best = (keyv, leaves)
        return best[1]
    pi = pts_int[idx]
    ax = int(np.argmax(pi.max(0) - pi.min(0)))
    l, r = split(idx, ax)
    return (_kd_split(pts, pts_int, l, depth - 1, gsel_count)
            + _kd_split(pts, pts_int, r, depth - 1, gsel_count))


def _prepare(inputs):
    """Host-side O(P log P + blocks*G) prep: blocking, gaussian selection,
    fp16 feature/coefficient matrices."""
    pts = np.ascontiguousarray(np.asarray(inputs["pts"], dtype=np.float32))
    means3D = np.ascontiguousarray(np.asarray(inputs["means3D"], dtype=np.float32))
    opac = np.asarray(inputs["opacities"], dtype=np.float32)
    scales = np.asarray(inputs["scales"], dtype=np.float32)
    cov3D = np.asarray(inputs["cov3D"], dtype=np.float32)
    pc_min = np.asarray(inputs["pc_min"], dtype=np.float32)

    P = pts.shape[0]
    G = means3D.shape[0]
    C = opac.shape[1]
    NBLK = P // BLK
    assert NBLK == N_CORES * NSLICE, (P, BLK)

    # integer voxel quantities, identical fp32 arithmetic to the reference
    pts_int = np.floor((pts - pc_min[None, :]) / GRID).astype(np.int32)
    means_int = np.floor((means3D - pc_min[None, :]) / GRID).astype(np.int32)
    radii = np.ceil(scales.max(-1) * SCALE_MULT / GRID).astype(np.int32)
    a_diag = np.stack([cov3D[:, 0, 0], cov3D[:, 1, 1], cov3D[:, 2, 2]], 1).astype(np.float64)
    off = cov3D.reshape(G, 9)[:, [1, 5, 2]]
    assert np.abs(off).max() == 0.0, "non-diagonal cov3D unsupported by this kernel"

    def gsel_exact(ix):
        """Gaussians with at least one of the points inside their voxel box."""
        pi = pts_int[ix]
        cand = np.where(((means_int >= pi.min(0) - radii[:, None])
                         & (means_int <= pi.max(0) + radii[:, None])).all(1))[0]
        within = (np.abs(pi[:, None, :] - means_int[None, cand, :])
                  <= radii[cand][None, :, None]).all(-1).any(0)
        return cand[within]

    blocks = _kd_split(pts, pts_int, np.arange(P), 6,
                       lambda ix: len(gsel_exact(ix)))

    # per block: compressed one-hot rows + exact gaussian selection
    binfo = []
    for b in blocks:
        pi = pts_int[b]
        uniq = [np.unique(pi[:, a]) for a in range(3)]
        nrows = sum(len(u) for u in uniq)
        binfo.append((b, uniq, gsel_exact(b)))
        assert NPOLY + nrows <= 128, f"row budget exceeded: {NPOLY + nrows}"

    KT = max(NPOLY + sum(len(u) for u in info[1]) for info in binfo)
    chunks = [max(1, int(np.ceil(len(info[2]) / 128))) for info in binfo]

    nrows_all = [NPOLY + sum(len(u) for u in info[1]) for info in binfo]
    if all(c == 1 for c in chunks):
        # one chunk per block: deal the globally lowest-row blocks into the
        # hot slots (4 per core) so HR (the critical first DMA's row count)
        # is the 32nd-lowest row count instead of an assignment accident
        order = np.argsort(nrows_all, kind="stable")
        core_blocks = [[] for _ in range(N_CORES)]
        for k, bi in enumerate(order):
            core_blocks[k % N_CORES].append(int(bi))
        core_chunks = [NSLICE] * N_CORES
    else:
        # greedy block->core assignment balancing chunk counts
        order = np.argsort(-np.asarray(chunks), kind="stable")
        core_blocks = [[] for _ in range(N_CORES)]
        core_chunks = [0] * N_CORES
        for bi in order:
            ci = min((c for c in range(N_CORES) if len(core_blocks[c]) < NSLICE),
                     key=lambda c: core_chunks[c])
            core_blocks[ci].append(bi)
            core_chunks[ci] += chunks[bi]
    NJOBS = max(core_chunks)
    assert NJOBS >= NSLICE

    HOT = 4 * 128 + 4 * 256
    TOTC = NJOBS * 128 + NSLICE * BLK

    def col_w(j):
        return j * 128 if j < 4 else HOT + (j - 4) * 128

    def col_f(s):
        return 512 + s * BLK if s < 4 else HOT + (NJOBS - 4) * 128 + (s - 4) * BLK

    in_maps = []
    perm = np.empty((N_CORES, NSLICE * BLK), np.int64)
    nrows_of = {bi: NPOLY + sum(len(u) for u in binfo[bi][1]) for bi in range(len(binfo))}
    HR = 0
    for ci in range(N_CORES):
        # low-row blocks first (shrinks the hot DMA), multi-chunk block last
        blks = sorted(core_blocks[ci], key=lambda bi: (chunks[bi], nrows_of[bi]))
        core_blocks[ci] = blks
        HR = max(HR, max(nrows_of[bi] for bi in blks[:4]))
    for ci in range(N_CORES):
        blks = core_blocks[ci]
        assert sum(c > 1 for c in (chunks[bi] for bi in blks[:-1])) == 0, \
            "more than one multi-chunk block on a core"
        FW = np.zeros((KT, TOTC), np.float16)
        OPA = np.zeros((128, NJOBS * C), np.float16)
        job = 0
        for si, bi in enumerate(blks):
            b, uniq, sel = binfo[bi]
            perm[ci, si * BLK:(si + 1) * BLK] = b
            pi = pts_int[b]
            lo = pi.min(0)
            hi = pi.max(0)
            cen = (lo + hi + 1).astype(np.float64) * (0.5 * float(GRID))
            p64 = pts[b].astype(np.float64) - cen

            # --- features for this slice ---
            F = np.zeros((KT, BLK), np.float16)
            r = 0
            for ax in range(3):
                qh, ql = _hilo(p64[:, ax] ** 2)
                xh, xl = _hilo(p64[:, ax])
                F[r], F[r + 1], F[r + 2] = qh, ql, qh
                F[r + 3], F[r + 4], F[r + 5] = xh, xl, xh
                r += 6
            F[18] = np.float16(1.0)
            F[19] = np.float16(1.0)
            offs = []
            racc = NPOLY
            for ax in range(3):
                offs.append(racc)
                racc += len(uniq[ax])
            tcol = np.arange(BLK)
            for ax in range(3):
                rank = np.searchsorted(uniq[ax], pi[:, ax])
                F[offs[ax] + rank, tcol] = np.float16(1.0)
            FW[:, col_f(si):col_f(si) + BLK] = F

            # --- per-chunk gaussian coefficients ---
            nch = chunks[bi]
            for ch in range(nch):
                gsel = sel[ch * 128:(ch + 1) * 128]
                gl = len(gsel)
                m64 = means3D[gsel].astype(np.float64) - cen
                a = a_diag[gsel]
                W = np.zeros((KT, 128), np.float16)
                r = 0
                for ax in range(3):
                    wah, wal = _hilo(-0.5 * a[:, ax])
                    wbh, wbl = _hilo(a[:, ax] * m64[:, ax])
                    W[r, :gl], W[r + 1, :gl], W[r + 2, :gl] = wah, wah, wal
                    W[r + 3, :gl], W[r + 4, :gl], W[r + 5, :gl] = wbh, wbh, wbl
                    r += 6
                ch_, cl_ = _hilo(-0.5 * (a * m64 ** 2).sum(1))
                W[18, :gl], W[19, :gl] = ch_, cl_
                for ax in range(3):
                    u = uniq[ax]
                    box = ((u[:, None] >= (means_int[gsel, ax] - radii[gsel])[None, :])
                           & (u[:, None] <= (means_int[gsel, ax] + radii[gsel])[None, :]))
                    W[offs[ax]:offs[ax] + len(u), :gl] = np.where(
                        box, np.float16(0.0), np.float16(-MPEN))
                if ch == 0:
                    assert job == si, (job, si)
                # jobs 0..7 occupy slots 0..7 (slice order); extra chunks of
                # the last block (slice 7) land at slots 8..
                FW[:, col_w(job):col_w(job) + 128] = W
                OPA[:gl, job * C:(job + 1) * C] = opac[gsel].astype(np.float16)
                job += 1
        in_maps.append({"fw": FW, "opa": OPA})

    return in_maps, perm, (P, KT, NJOBS, C, HR)


def _run(inputs, trace=False, **run_kwargs):
    in_maps, perm, (P, KT, NJOBS, C, HR) = _prepare(inputs)
    key = (KT, NJOBS, C, HR)
    if key not in _nc_cache:
        nc = _build_bass(KT, NJOBS, C, HR)
        _nc_cache[key] = nc
    nc = _nc_cache[key]
    try:
        res = run_bass_kernel_spmd(
            nc, in_maps, core_ids=list(range(N_CORES)), trace=trace, **run_kwargs
        )
    except ModuleNotFoundError:
        res = run_bass_kernel_spmd(
            nc, in_maps, core_ids=list(range(N_CORES)), trace=False, **run_kwargs
        )
    out = np.empty((P, C), np.float32)
    for ci in range(N_CORES):
        out[perm[ci]] = res.results[ci]["out"].T.astype(np.float32)
    return out, res


def kernel(**inputs):
    return _run(inputs)[0]
